# revision 7
# baseline (speedup 1.0000x reference)
"""Trainium2 Bass kernel for nn_CrossAttention_82429012345074.

8-head self-attention, B=2, N=4096, d_model=512, 8 heads x 64 dim.
One head per NeuronCore (8 heads / 8 cores); host sums the 8 partial
[tok, 512] outputs + bias.

v2: fp8e4m3 DoubleRow matmuls (0.5 cycles/row) for the attention core.
The HW activity governor (HAM) duty-throttles a continuously-busy PE to
~50%, so sustained matmul throughput is capped near 1 row/ns — halving
row-cycles via fp8 DoubleRow is the main lever.

Precision plan (tolerance 2e-2; bf16 baseline measured 1.1e-3):
- QK^T in fp8-DR with the two k-tile slots = feature halves (32p x 2).
  fp8 quantization of q/k perturbs scores ~2.6% rel -> softmax weights
  ~0.7% rms. OK.
- PV uses p' = p - 1 (expm1 via ACT exp + DVE/GPSIMD fused subtract-cast
  to fp8). rms(p') ~ 0.21 vs rms(p) ~ 1.04, so fp8 quantization of p'
  gives ~5x smaller absolute weight error than quantizing p. The missing
  +1 terms (sum_j v_j) are added back exactly via a host-precomputed
  svwo row folded into the output projection; the denominator gets +4096.
- V itself fp8: its error enters only through p'-weighted terms (5x
  attenuated). V/out projections read x8/oN via bf16-precision paths
  where it matters (out projection stays bf16).

Per-core device kernel:
  q8/k8 = fp8 casts of (Wq/Wk @ x) in [32, 2(slot), tok] layout
  v8    = fp8 cast of (x @ Wv.T) in [128, pair, 2(slot), 65] (+ones col)
  s     = DR-matmul(k8-block, q8-group)          # [128, 512] PSUM
  p     = exp(s * scale)                         # ACT, PSUM->SBUF bf16
  p'8   = (p - 1) -> fp8                         # DVE/GPSIMD alternating
  o    += DR-matmul(v8-pair, p'8-pair)           # [65, 512] PSUM accum
  fp    = [oN | 1].T @ [wo ; svwo_b]             # 65-row bf16 projection
  out   = fp * (1 / (o[64] + 4096))              # per-token scalar
"""

import sys

sys.path.insert(0, "/opt/trn_rl_repo")

import numpy as np
import ml_dtypes

B, N, D, H, DH = 2, 4096, 512, 8, 64
TOK = B * N            # 8192
NQ = 512               # query-group width
NCH = D // 128         # 4 feature chunks of x
NJB = N // 128         # 32 key blocks per batch
NTB = TOK // 128       # 64 token blocks
JGS = [3] * 10 + [2]   # key-blocks per exp() chunk (sum = 32)
VW = 128               # vP8 row width: 64 v cols + ones col + zero pad to a
                       # legal DR stationary tile width (must be 32/64/128)
SCALE = DH ** -0.5


def build_bass():
    from contextlib import ExitStack

    import concourse.bass as bass
    import concourse.mybir as mybir
    import concourse.tile as tile
    from concourse import bacc

    f32 = mybir.dt.float32
    bf16 = mybir.dt.bfloat16
    f8 = mybir.dt.float8e4
    EXP = mybir.ActivationFunctionType.Exp
    DR = mybir.MatmulPerfMode.DoubleRow

    nc = bacc.Bacc("TRN2", target_bir_lowering=False, num_devices=8)
    x8_d = nc.dram_tensor("x8", [NCH, 128, TOK], f8, kind="ExternalInput")
    wq8_d = nc.dram_tensor("wq8", [128, 2, 2, DH], f8, kind="ExternalInput")
    wk8_d = nc.dram_tensor("wk8", [128, 2, 2, DH], f8, kind="ExternalInput")
    wv8_d = nc.dram_tensor("wv8", [128, 2, 2, DH], f8, kind="ExternalInput")
    wo65_d = nc.dram_tensor("wo65", [DH + 1, B, D], bf16, kind="ExternalInput")
    out_d = nc.dram_tensor("out", [TOK, D], f32, kind="ExternalOutput")

    with tile.TileContext(nc) as tc, ExitStack() as ctx:
        const = ctx.enter_context(tc.tile_pool(name="const", bufs=1))
        sb_p = ctx.enter_context(tc.tile_pool(name="sb_p", bufs=3))
        sb_io = ctx.enter_context(tc.tile_pool(name="sb_io", bufs=3))
        ps_s = ctx.enter_context(tc.tile_pool(name="ps_s", bufs=2, space="PSUM"))
        ps_sm = ctx.enter_context(tc.tile_pool(name="ps_sm", bufs=2, space="PSUM"))

        # Long-lived SBUF tensors
        x8 = const.tile([128, NCH, TOK], f8, name="x8")
        q8 = const.tile([32, 2, TOK], f8, name="q8")   # feat d -> (d%32, d//32)
        k8 = const.tile([32, 2, TOK], f8, name="k8")
        p8 = const.tile([128, NJB, NQ], f8, name="p8")  # p-1 for current group
        vP8 = const.tile([128, NTB // 2, 2, VW], f8, name="vP8")
        oN = const.tile([DH + 1, TOK], bf16, name="oN")
        wq8 = const.tile([128, 2, 2, DH], f8, name="wq8")
        wk8 = const.tile([128, 2, 2, DH], f8, name="wk8")
        wv8 = const.tile([128, 2, 2, DH], f8, name="wv8")
        wo65 = const.tile([DH + 1, B, D], bf16, name="wo65")

        nc.vector.memset(vP8[:, :, :, DH : DH + 1], 1.0)
        nc.vector.memset(vP8[:, :, :, DH + 1 : VW], 0.0)
        nc.vector.memset(oN[DH : DH + 1, :], 1.0)
        ones1 = const.tile([1, 1], f32, name="ones1")
        nc.vector.memset(ones1, 1.0)

        nc.sync.dma_start(out=wq8, in_=wq8_d[:])
        nc.sync.dma_start(out=wk8, in_=wk8_d[:])
        nc.sync.dma_start(out=wv8, in_=wv8_d[:])
        nc.sync.dma_start(out=wo65, in_=wo65_d[:])

        # Phase 0: load host-pre-transposed fp8 x chunks
        for c in range(NCH):
            for tt in range(TOK // 2048):
                t0 = tt * 2048
                nc.sync.dma_start(
                    out=x8[:, c, t0 : t0 + 2048],
                    in_=x8_d[c, :, t0 : t0 + 2048],
                )

        # Phase 1: projections (fp8 DoubleRow over feature-chunk pairs)
        for g in range(TOK // NQ):
            t0 = g * NQ
            qp = ps_s.tile([64, NQ], f32, tag="s", name="qp")
            for cp in range(2):
                nc.tensor.matmul(
                    qp, lhsT=wq8[:, cp, :, :],
                    rhs=x8[:, 2 * cp : 2 * cp + 2, t0 : t0 + NQ],
                    start=(cp == 0), stop=(cp == 1), perf_mode=DR,
                )
            nc.vector.tensor_copy(out=q8[:, 0, t0 : t0 + NQ], in_=qp[0:32, :])
            nc.vector.tensor_copy(out=q8[:, 1, t0 : t0 + NQ], in_=qp[32:64, :])

            kp = ps_s.tile([64, NQ], f32, tag="s", name="kp")
            for cp in range(2):
                nc.tensor.matmul(
                    kp, lhsT=wk8[:, cp, :, :],
                    rhs=x8[:, 2 * cp : 2 * cp + 2, t0 : t0 + NQ],
                    start=(cp == 0), stop=(cp == 1), perf_mode=DR,
                )
            nc.vector.tensor_copy(out=k8[:, 0, t0 : t0 + NQ], in_=kp[0:32, :])
            nc.vector.tensor_copy(out=k8[:, 1, t0 : t0 + NQ], in_=kp[32:64, :])

            for t in range(NQ // 128):
                tb = g * (NQ // 128) + t
                vp = ps_sm.tile([128, DH], f32, tag="o", name="vp")
                for cp in range(2):
                    nc.tensor.matmul(
                        vp,
                        lhsT=x8[:, 2 * cp : 2 * cp + 2, tb * 128 : tb * 128 + 128],
                        rhs=wv8[:, cp, :, :],
                        start=(cp == 0), stop=(cp == 1), perf_mode=DR,
                    )
                nc.scalar.activation(
                    out=vP8[:, tb // 2, tb % 2, 0:DH], in_=vp,
                    func=mybir.ActivationFunctionType.Copy,
                )

        # Phase 2+3: attention + output projection.
        def emit_denT(pq0, pden, pb):
            denT = ps_sm.tile([128, 4], f32, tag="o", name="denT")
            for t in range(NQ // 128):
                nc.tensor.matmul(
                    denT[:, t : t + 1],
                    lhsT=pden[0:1, t * 128 : (t + 1) * 128],
                    rhs=ones1, start=True, stop=True,
                )
            recT = sb_io.tile([128, 4], f32, name="recT")
            nc.vector.reciprocal(recT, denT)
            return recT

        def emit_fp_one(pq0, pb, t, recT):
            tt0 = pq0 + t * 128
            fp = ps_sm.tile([128, D], f32, tag="o", name="fp")
            nc.tensor.matmul(
                fp, lhsT=oN[:, tt0 : tt0 + 128], rhs=wo65[:, pb, :],
                start=True, stop=True,
            )
            ob = sb_io.tile([128, D], f32, name="ob")
            nc.vector.tensor_scalar_mul(ob, in0=fp, scalar1=recT[:, t : t + 1])
            nc.sync.dma_start(out=out_d[tt0 : tt0 + 128, :], in_=ob)

        pending = None  # (q0, den tile, batch) of the previous group
        chunk_id = 0
        for b in range(B):
            for qg in range(N // NQ):
                q0 = b * N + qg * NQ
                o = ps_sm.tile([VW, NQ], f32, tag="o", name="o")
                jb = 0
                recT = None
                emitted_pairs = 0

                def flush_pairs(upto):
                    nonlocal emitted_pairs
                    while emitted_pairs < upto:
                        pg = emitted_pairs
                        nc.tensor.matmul(
                            o,
                            lhsT=vP8[:, b * (NJB // 2) + pg, :, :],
                            rhs=p8[:, 2 * pg : 2 * pg + 2, :],
                            start=(pg == 0), stop=(pg == NJB // 2 - 1),
                            perf_mode=DR,
                        )
                        emitted_pairs += 1

                for gi, gsz in enumerate(JGS):
                    s = ps_s.tile([128, 3, NQ], f32, tag="s", name="s")
                    for i in range(gsz):
                        j0 = b * N + (jb + i) * 128
                        nc.tensor.matmul(
                            s[:, i, :], lhsT=k8[:, :, j0 : j0 + 128],
                            rhs=q8[:, :, q0 : q0 + NQ], start=True, stop=True,
                            perf_mode=DR,
                        )
                    p = sb_p.tile([128, 3, NQ], bf16, name="p")
                    nc.scalar.activation(
                        out=p[:, 0:gsz, :], in_=s[:, 0:gsz, :], func=EXP, scale=SCALE
                    )
                    eng = nc.vector if chunk_id % 2 == 0 else nc.gpsimd
                    eng.tensor_scalar_sub(
                        p8[:, jb : jb + gsz, :], in0=p[:, 0:gsz, :], scalar1=1.0
                    )
                    chunk_id += 1
                    # PV for pairs fully cast by PREVIOUS chunks (defer 1 chunk
                    # so exp/cast of this chunk overlap PV of the last one).
                    flush_pairs(jb // 2)
                    jb += gsz
                    # previous group's epilogue, spread across this loop
                    if pending is not None:
                        if gi == 1:
                            recT = emit_denT(*pending)
                        elif gi in (3, 5, 7, 9):
                            emit_fp_one(pending[0], pending[2], (gi - 3) // 2, recT)
                flush_pairs(NJB // 2)

                nc.vector.tensor_copy(out=oN[0:DH, q0 : q0 + NQ], in_=o[0:DH, :])
                den = sb_io.tile([1, NQ], f32, name="den")
                nc.vector.tensor_scalar_add(den, in0=o[DH : DH + 1, :], scalar1=float(N))
                pending = (q0, den, b)

        recT = emit_denT(*pending)
        for t in range(NQ // 128):
            emit_fp_one(pending[0], pending[2], t, recT)

    nc.compile()
    return nc


def make_in_maps(x, Wq, Wk, Wv, Wo):
    f8 = ml_dtypes.float8_e4m3
    bf16 = ml_dtypes.bfloat16
    x2 = x.reshape(TOK, D)
    # x transposed to [feat, tok] and chunked: [NCH, 128, TOK]
    x8 = np.ascontiguousarray(x2.T.reshape(NCH, 128, TOK)).astype(f8)
    # per-batch token sums for the svwo correction row
    sumx = x.reshape(B, N, D).sum(axis=1)  # [B, D] fp32

    def wsplit(W, h):
        # [p, cp, i, d] = W.T[(2cp+i)*128+p, d] for the head's 64 columns
        wt = np.ascontiguousarray(W[h * DH : (h + 1) * DH, :].T)  # [512, 64]
        return np.ascontiguousarray(
            wt.reshape(2, 2, 128, DH).transpose(2, 0, 1, 3)
        ).astype(f8)

    in_maps = []
    for h in range(H):
        sl = slice(h * DH, (h + 1) * DH)
        wo = np.ascontiguousarray(Wo[:, sl].T)  # [64, 512]
        sumv = sumx @ Wv[sl, :].T               # [B, 64]
        svwo = sumv @ wo                        # [B, 512]
        wo65 = np.empty((DH + 1, B, D), dtype=np.float32)
        wo65[0:DH] = wo[:, None, :]
        wo65[DH] = svwo
        in_maps.append(
            {
                "x8": x8,
                "wq8": wsplit(Wq, h),
                "wk8": wsplit(Wk, h),
                "wv8": wsplit(Wv, h),
                "wo65": wo65.astype(bf16),
            }
        )
    return in_maps


def _install_ntff_shim():
    """The axon boot skips registering the NTFF profile hook when the image's
    antenv lacks axon_hooks; register an equivalent shim so trace=True works."""
    import types

    if "antenv.axon_hooks" in sys.modules:
        return
    try:
        from trn_agent_boot.trn_boot import _ntff_profile_via_ctypes

        hook = _ntff_profile_via_ctypes("/opt/axon/libaxon_pjrt.so")
    except Exception:
        hook = None
    mod = types.ModuleType("antenv.axon_hooks")
    mod.get_axon_ntff_profile_hook = lambda: hook
    sys.modules["antenv.axon_hooks"] = mod


def run(x, Wq, Wk, Wv, Wo, bo, trace=False):
    from concourse.bass_utils import run_bass_kernel_spmd

    if trace:
        _install_ntff_shim()

    nc = build_bass()
    in_maps = make_in_maps(x, Wq, Wk, Wv, Wo)
    res = run_bass_kernel_spmd(nc, in_maps, core_ids=list(range(H)), trace=trace)
    acc = np.zeros((TOK, D), dtype=np.float32)
    for r in res.results:
        acc += r["out"]
    acc += np.asarray(bo, dtype=np.float32)[None, :]
    return acc.reshape(B, N, D), res


def kernel(x, Wq, Wk, Wv, Wo, bo):
    out, _ = run(
        np.asarray(x, dtype=np.float32),
        np.asarray(Wq, dtype=np.float32),
        np.asarray(Wk, dtype=np.float32),
        np.asarray(Wv, dtype=np.float32),
        np.asarray(Wo, dtype=np.float32),
        np.asarray(bo, dtype=np.float32),
    )
    return out


# revision 8
# speedup vs baseline: 2.3844x; 2.3844x over previous
"""Trainium2 Bass kernel for nn_CrossAttention_82429012345074.

8-head self-attention, B=2, N=4096, d_model=512, 8 heads x 64 dim.
One head per NeuronCore (8 heads / 8 cores); host sums the 8 partial
[tok, 512] outputs + bias.

v2: fp8e4m3 DoubleRow matmuls (0.5 cycles/row) for the attention core.
The HW activity governor (HAM) duty-throttles a continuously-busy PE to
~50%, so sustained matmul throughput is capped near 1 row/ns — halving
row-cycles via fp8 DoubleRow is the main lever.

Precision plan (tolerance 2e-2; bf16 baseline measured 1.1e-3):
- QK^T in fp8-DR with the two k-tile slots = feature halves (32p x 2).
  fp8 quantization of q/k perturbs scores ~2.6% rel -> softmax weights
  ~0.7% rms. OK.
- PV uses p' = p - 1 (expm1 via ACT exp + DVE/GPSIMD fused subtract-cast
  to fp8). rms(p') ~ 0.21 vs rms(p) ~ 1.04, so fp8 quantization of p'
  gives ~5x smaller absolute weight error than quantizing p. The missing
  +1 terms (sum_j v_j) are added back exactly via a host-precomputed
  svwo row folded into the output projection; the denominator gets +4096.
- V itself fp8: its error enters only through p'-weighted terms (5x
  attenuated). V/out projections read x8/oN via bf16-precision paths
  where it matters (out projection stays bf16).

Per-core device kernel:
  q8/k8 = fp8 casts of (Wq/Wk @ x) in [32, 2(slot), tok] layout
  v8    = fp8 cast of (x @ Wv.T) in [128, pair, 2(slot), 65] (+ones col)
  s     = DR-matmul(k8-block, q8-group)          # [128, 512] PSUM
  p     = exp(s * scale)                         # ACT, PSUM->SBUF bf16
  p'8   = (p - 1) -> fp8                         # DVE/GPSIMD alternating
  o    += DR-matmul(v8-pair, p'8-pair)           # [65, 512] PSUM accum
  fp    = [oN | 1].T @ [wo ; svwo_b]             # 65-row bf16 projection
  out   = fp * (1 / (o[64] + 4096))              # per-token scalar
"""

import sys

sys.path.insert(0, "/opt/trn_rl_repo")

import numpy as np
import ml_dtypes

B, N, D, H, DH = 2, 4096, 512, 8, 64
TOK = B * N            # 8192
NQ = 512               # query-group width
NCH = D // 128         # 4 feature chunks of x
NJB = N // 128         # 32 key blocks per batch
NTB = TOK // 128       # 64 token blocks
JGS = [3] * 10 + [2]   # key-blocks per exp() chunk (sum = 32)
VW = 128               # vP8 row width: 64 v cols + ones col + zero pad to a
                       # legal DR stationary tile width (must be 32/64/128)
SCALE = DH ** -0.5


def build_bass():
    from contextlib import ExitStack

    import concourse.bass as bass
    import concourse.mybir as mybir
    import concourse.tile as tile
    from concourse import bacc

    f32 = mybir.dt.float32
    bf16 = mybir.dt.bfloat16
    f8 = mybir.dt.float8e4
    EXP = mybir.ActivationFunctionType.Exp
    DR = mybir.MatmulPerfMode.DoubleRow

    nc = bacc.Bacc("TRN2", target_bir_lowering=False, num_devices=8)
    x8_d = nc.dram_tensor("x8", [NCH, 128, TOK], f8, kind="ExternalInput")
    wq8_d = nc.dram_tensor("wq8", [128, 2, 2, DH], f8, kind="ExternalInput")
    wk8_d = nc.dram_tensor("wk8", [128, 2, 2, DH], f8, kind="ExternalInput")
    wv8_d = nc.dram_tensor("wv8", [128, 2, 2, DH], f8, kind="ExternalInput")
    wo65_d = nc.dram_tensor("wo65", [DH + 1, B, D], bf16, kind="ExternalInput")
    out_d = nc.dram_tensor("out", [TOK, D], f32, kind="ExternalOutput")

    with tile.TileContext(nc) as tc, ExitStack() as ctx:
        const = ctx.enter_context(tc.tile_pool(name="const", bufs=1))
        sb_p = ctx.enter_context(tc.tile_pool(name="sb_p", bufs=3))
        sb_io = ctx.enter_context(tc.tile_pool(name="sb_io", bufs=3))
        ps_s = ctx.enter_context(tc.tile_pool(name="ps_s", bufs=2, space="PSUM"))
        ps_sm = ctx.enter_context(tc.tile_pool(name="ps_sm", bufs=2, space="PSUM"))

        # Long-lived SBUF tensors
        x8 = const.tile([128, NCH, TOK], f8, name="x8")
        q8 = const.tile([32, 2, TOK], f8, name="q8")   # feat d -> (d%32, d//32)
        k8 = const.tile([32, 2, TOK], f8, name="k8")
        p8 = const.tile([128, NJB, NQ], f8, name="p8")  # p-1 for current group
        vP8 = const.tile([128, NTB // 2, 2, VW], f8, name="vP8")
        oN = const.tile([DH + 1, TOK], bf16, name="oN")
        wq8 = const.tile([128, 2, 2, DH], f8, name="wq8")
        wk8 = const.tile([128, 2, 2, DH], f8, name="wk8")
        wv8 = const.tile([128, 2, 2, DH], f8, name="wv8")
        wo65 = const.tile([DH + 1, B, D], bf16, name="wo65")

        nc.vector.memset(vP8[:, :, :, DH : DH + 1], 1.0)
        nc.vector.memset(vP8[:, :, :, DH + 1 : VW], 0.0)
        nc.vector.memset(oN[DH : DH + 1, :], 1.0)
        ones1 = const.tile([1, 1], f32, name="ones1")
        nc.vector.memset(ones1, 1.0)

        nc.sync.dma_start(out=wq8, in_=wq8_d[:])
        nc.sync.dma_start(out=wk8, in_=wk8_d[:])
        nc.sync.dma_start(out=wv8, in_=wv8_d[:])
        nc.sync.dma_start(out=wo65, in_=wo65_d[:])

        # Phase 0: load host-pre-transposed fp8 x chunks
        for c in range(NCH):
            for tt in range(TOK // 2048):
                t0 = tt * 2048
                nc.sync.dma_start(
                    out=x8[:, c, t0 : t0 + 2048],
                    in_=x8_d[c, :, t0 : t0 + 2048],
                )

        # Phase 1: projections (fp8 DoubleRow over feature-chunk pairs)
        for g in range(TOK // NQ):
            t0 = g * NQ
            qp = ps_s.tile([64, NQ], f32, tag="s", name="qp")
            for cp in range(2):
                nc.tensor.matmul(
                    qp, lhsT=wq8[:, cp, :, :],
                    rhs=x8[:, 2 * cp : 2 * cp + 2, t0 : t0 + NQ],
                    start=(cp == 0), stop=(cp == 1), perf_mode=DR,
                )
            nc.vector.tensor_copy(out=q8[:, 0, t0 : t0 + NQ], in_=qp[0:32, :])
            nc.vector.tensor_copy(out=q8[:, 1, t0 : t0 + NQ], in_=qp[32:64, :])

            kp = ps_s.tile([64, NQ], f32, tag="s", name="kp")
            for cp in range(2):
                nc.tensor.matmul(
                    kp, lhsT=wk8[:, cp, :, :],
                    rhs=x8[:, 2 * cp : 2 * cp + 2, t0 : t0 + NQ],
                    start=(cp == 0), stop=(cp == 1), perf_mode=DR,
                )
            nc.vector.tensor_copy(out=k8[:, 0, t0 : t0 + NQ], in_=kp[0:32, :])
            nc.vector.tensor_copy(out=k8[:, 1, t0 : t0 + NQ], in_=kp[32:64, :])

            for t in range(NQ // 128):
                tb = g * (NQ // 128) + t
                vp = ps_sm.tile([128, DH], f32, tag="o", name="vp")
                for cp in range(2):
                    nc.tensor.matmul(
                        vp,
                        lhsT=x8[:, 2 * cp : 2 * cp + 2, tb * 128 : tb * 128 + 128],
                        rhs=wv8[:, cp, :, :],
                        start=(cp == 0), stop=(cp == 1), perf_mode=DR,
                    )
                nc.scalar.activation(
                    out=vP8[:, tb // 2, tb % 2, 0:DH], in_=vp,
                    func=mybir.ActivationFunctionType.Copy,
                )

        # Phase 2+3: attention + output projection.
        def emit_denT(pq0, pden, pb):
            denT = ps_sm.tile([128, 4], f32, tag="o", name="denT")
            for t in range(NQ // 128):
                nc.tensor.matmul(
                    denT[:, t : t + 1],
                    lhsT=pden[0:1, t * 128 : (t + 1) * 128],
                    rhs=ones1, start=True, stop=True,
                )
            recT = sb_io.tile([128, 4], f32, name="recT")
            nc.vector.reciprocal(recT, denT)
            return recT

        def emit_fp_one(pq0, pb, t, recT):
            tt0 = pq0 + t * 128
            fp = ps_sm.tile([128, D], f32, tag="o", name="fp")
            nc.tensor.matmul(
                fp, lhsT=oN[:, tt0 : tt0 + 128], rhs=wo65[:, pb, :],
                start=True, stop=True,
            )
            ob = sb_io.tile([128, D], f32, name="ob")
            nc.vector.tensor_scalar_mul(ob, in0=fp, scalar1=recT[:, t : t + 1])
            nc.sync.dma_start(out=out_d[tt0 : tt0 + 128, :], in_=ob)

        pending = None  # (q0, den tile, batch) of the previous group
        chunk_id = 0
        for b in range(B):
            for qg in range(N // NQ):
                q0 = b * N + qg * NQ
                o = ps_sm.tile([VW, NQ], f32, tag="o", name="o")
                jb = 0
                recT = None
                emitted_pairs = 0

                def flush_pairs(upto):
                    nonlocal emitted_pairs
                    while emitted_pairs < upto:
                        pg = emitted_pairs
                        nc.tensor.matmul(
                            o,
                            lhsT=vP8[:, b * (NJB // 2) + pg, :, :],
                            rhs=p8[:, 2 * pg : 2 * pg + 2, :],
                            start=(pg == 0), stop=(pg == NJB // 2 - 1),
                            perf_mode=DR,
                        )
                        emitted_pairs += 1

                for gi, gsz in enumerate(JGS):
                    s = ps_s.tile([128, 3, NQ], f32, tag="s", name="s")
                    for i in range(gsz):
                        j0 = b * N + (jb + i) * 128
                        nc.tensor.matmul(
                            s[:, i, :], lhsT=k8[:, :, j0 : j0 + 128],
                            rhs=q8[:, :, q0 : q0 + NQ], start=True, stop=True,
                            perf_mode=DR,
                        )
                    p = sb_p.tile([128, 3, NQ], bf16, name="p")
                    nc.scalar.activation(
                        out=p[:, 0:gsz, :], in_=s[:, 0:gsz, :], func=EXP, scale=SCALE
                    )
                    # p-1 in bf16 (2-byte DVE fast path), then the fp8
                    # conversion via TensorCopy (dedicated convert path;
                    # a fused TensorScalar->fp8 is microcoded ~10x slower).
                    pm = sb_p.tile([128, 3, NQ], bf16, name="pm")
                    nc.vector.tensor_scalar_sub(
                        pm[:, 0:gsz, :], in0=p[:, 0:gsz, :], scalar1=1.0
                    )
                    ceng = nc.vector if chunk_id % 3 == 0 else nc.gpsimd
                    ceng.tensor_copy(
                        out=p8[:, jb : jb + gsz, :], in_=pm[:, 0:gsz, :]
                    )
                    chunk_id += 1
                    # PV for pairs fully cast by PREVIOUS chunks (defer 1 chunk
                    # so exp/cast of this chunk overlap PV of the last one).
                    flush_pairs(jb // 2)
                    jb += gsz
                    # previous group's epilogue, spread across this loop
                    if pending is not None:
                        if gi == 1:
                            recT = emit_denT(*pending)
                        elif gi in (3, 5, 7, 9):
                            emit_fp_one(pending[0], pending[2], (gi - 3) // 2, recT)
                flush_pairs(NJB // 2)

                nc.vector.tensor_copy(out=oN[0:DH, q0 : q0 + NQ], in_=o[0:DH, :])
                den = sb_io.tile([1, NQ], f32, name="den")
                nc.vector.tensor_scalar_add(den, in0=o[DH : DH + 1, :], scalar1=float(N))
                pending = (q0, den, b)

        recT = emit_denT(*pending)
        for t in range(NQ // 128):
            emit_fp_one(pending[0], pending[2], t, recT)

    nc.compile()
    return nc


def make_in_maps(x, Wq, Wk, Wv, Wo):
    f8 = ml_dtypes.float8_e4m3
    bf16 = ml_dtypes.bfloat16
    x2 = x.reshape(TOK, D)
    # x transposed to [feat, tok] and chunked: [NCH, 128, TOK]
    x8 = np.ascontiguousarray(x2.T.reshape(NCH, 128, TOK)).astype(f8)
    # per-batch token sums for the svwo correction row
    sumx = x.reshape(B, N, D).sum(axis=1)  # [B, D] fp32

    def wsplit(W, h):
        # [p, cp, i, d] = W.T[(2cp+i)*128+p, d] for the head's 64 columns
        wt = np.ascontiguousarray(W[h * DH : (h + 1) * DH, :].T)  # [512, 64]
        return np.ascontiguousarray(
            wt.reshape(2, 2, 128, DH).transpose(2, 0, 1, 3)
        ).astype(f8)

    in_maps = []
    for h in range(H):
        sl = slice(h * DH, (h + 1) * DH)
        wo = np.ascontiguousarray(Wo[:, sl].T)  # [64, 512]
        sumv = sumx @ Wv[sl, :].T               # [B, 64]
        svwo = sumv @ wo                        # [B, 512]
        wo65 = np.empty((DH + 1, B, D), dtype=np.float32)
        wo65[0:DH] = wo[:, None, :]
        wo65[DH] = svwo
        in_maps.append(
            {
                "x8": x8,
                "wq8": wsplit(Wq, h),
                "wk8": wsplit(Wk, h),
                "wv8": wsplit(Wv, h),
                "wo65": wo65.astype(bf16),
            }
        )
    return in_maps


def _install_ntff_shim():
    """The axon boot skips registering the NTFF profile hook when the image's
    antenv lacks axon_hooks; register an equivalent shim so trace=True works."""
    import types

    if "antenv.axon_hooks" in sys.modules:
        return
    try:
        from trn_agent_boot.trn_boot import _ntff_profile_via_ctypes

        hook = _ntff_profile_via_ctypes("/opt/axon/libaxon_pjrt.so")
    except Exception:
        hook = None
    mod = types.ModuleType("antenv.axon_hooks")
    mod.get_axon_ntff_profile_hook = lambda: hook
    sys.modules["antenv.axon_hooks"] = mod


def run(x, Wq, Wk, Wv, Wo, bo, trace=False):
    from concourse.bass_utils import run_bass_kernel_spmd

    if trace:
        _install_ntff_shim()

    nc = build_bass()
    in_maps = make_in_maps(x, Wq, Wk, Wv, Wo)
    res = run_bass_kernel_spmd(nc, in_maps, core_ids=list(range(H)), trace=trace)
    acc = np.zeros((TOK, D), dtype=np.float32)
    for r in res.results:
        acc += r["out"]
    acc += np.asarray(bo, dtype=np.float32)[None, :]
    return acc.reshape(B, N, D), res


def kernel(x, Wq, Wk, Wv, Wo, bo):
    out, _ = run(
        np.asarray(x, dtype=np.float32),
        np.asarray(Wq, dtype=np.float32),
        np.asarray(Wk, dtype=np.float32),
        np.asarray(Wv, dtype=np.float32),
        np.asarray(Wo, dtype=np.float32),
        np.asarray(bo, dtype=np.float32),
    )
    return out


# revision 9
# speedup vs baseline: 3.1456x; 1.3192x over previous
"""Trainium2 Bass kernel for nn_CrossAttention_82429012345074.

8-head self-attention, B=2, N=4096, d_model=512, 8 heads x 64 dim.
One head per NeuronCore (8 heads / 8 cores); host sums the 8 partial
[tok, 512] outputs + bias.

v4: fp8e4m3 DoubleRow (0.5 cycles/row) for the QK^T matmuls and the
q/k projections; PV and the v/out projections stay bf16.

Why this split: the HW activity governor (HAM) duty-throttles a
continuously-busy PE to ~50%, so the fix is to cut PE cycles until the
PE runs below the governor threshold and ACT (exp, ~1 el/cycle,
unthrottled) becomes the steady-state limiter.
- QK^T in fp8-DR halves the dominant score matmul (contraction 64 = two
  32-partition k-tile slots). fp8 q/k only perturb softmax weights
  (~0.7% output error). Tolerance is 2e-2; bf16 baseline was 1.1e-3.
- PV must stay bf16: fp8 P or V injects their full relative
  quantization error (~1.3-1.8%) straight into the output, and the only
  fast fp8-producing op is TensorCopy (fused ALU+fp8-convert on DVE or
  GPSIMD is microcoded at ~11 ns/el, measured), so there is no cheap
  high-precision fp8 PV operand.

Per-core device kernel (fp32 PSUM accumulation everywhere):
  q8/k8 = fp8 casts of (Wq/Wk @ x8) in [32, 2(slot), tok] layout
  v     = xT.T @ Wv.T in bf16 (+ones column)     # [tok, 65]
  s     = DR-matmul(k8-block, q8-group)          # [128, 512] PSUM
  p     = exp(s * scale)                         # ACT, PSUM->SBUF bf16
  o    += vP.T @ p per key block                 # [65, 512] PSUM accum
  oN    = o[0:64] bf16; den row -> transpose -> reciprocal
  out   = (oN.T @ wo) * (1/den)                  # per-token scalar, DMA
"""

import sys

sys.path.insert(0, "/opt/trn_rl_repo")

import numpy as np
import ml_dtypes

B, N, D, H, DH = 2, 4096, 512, 8, 64
TOK = B * N            # 8192
NQ = 512               # query-group width
NCH = D // 128         # 4 feature chunks of x
NJB = N // 128         # 32 key blocks per batch
NTB = TOK // 128       # 64 token blocks
JGS = [3] * 10 + [2]   # key-blocks per exp() chunk (sum = 32)
SCALE = DH ** -0.5


def build_bass():
    from contextlib import ExitStack

    import concourse.bass as bass
    import concourse.mybir as mybir
    import concourse.tile as tile
    from concourse import bacc

    f32 = mybir.dt.float32
    bf16 = mybir.dt.bfloat16
    f8 = mybir.dt.float8e4
    EXP = mybir.ActivationFunctionType.Exp
    CPY = mybir.ActivationFunctionType.Copy
    DR = mybir.MatmulPerfMode.DoubleRow

    nc = bacc.Bacc("TRN2", target_bir_lowering=False, num_devices=8)
    x8_d = nc.dram_tensor("x8", [NCH, 128, TOK], f8, kind="ExternalInput")
    xb_d = nc.dram_tensor("xb", [NCH, 128, TOK], bf16, kind="ExternalInput")
    wq8_d = nc.dram_tensor("wq8", [128, 2, 2, DH], f8, kind="ExternalInput")
    wk8_d = nc.dram_tensor("wk8", [128, 2, 2, DH], f8, kind="ExternalInput")
    wv_d = nc.dram_tensor("wv", [D, DH], bf16, kind="ExternalInput")
    wo_d = nc.dram_tensor("wo", [DH, D], bf16, kind="ExternalInput")
    out_d = nc.dram_tensor("out", [TOK, D], f32, kind="ExternalOutput")

    with tile.TileContext(nc) as tc, ExitStack() as ctx:
        const = ctx.enter_context(tc.tile_pool(name="const", bufs=1))
        sb_p = ctx.enter_context(tc.tile_pool(name="sb_p", bufs=3))
        sb_io = ctx.enter_context(tc.tile_pool(name="sb_io", bufs=3))
        ps_s = ctx.enter_context(tc.tile_pool(name="ps_s", bufs=2, space="PSUM"))
        ps_sm = ctx.enter_context(tc.tile_pool(name="ps_sm", bufs=2, space="PSUM"))

        # Long-lived SBUF tensors
        x8 = const.tile([128, NCH, TOK], f8, name="x8")
        xb = const.tile([128, NCH, TOK], bf16, name="xb")
        q8 = const.tile([32, 2, TOK], f8, name="q8")   # feat d -> (d%32, d//32)
        k8 = const.tile([32, 2, TOK], f8, name="k8")
        vP = const.tile([128, NTB, DH + 1], bf16, name="vP")   # v blocks + ones col
        oN = const.tile([64, TOK], bf16, name="oN")            # unnormalized attn out^T
        wq8 = const.tile([128, 2, 2, DH], f8, name="wq8")
        wk8 = const.tile([128, 2, 2, DH], f8, name="wk8")
        wv = const.tile([128, NCH, DH], bf16, name="wv")
        wo = const.tile([64, D], bf16, name="wo")

        nc.vector.memset(vP[:, :, DH : DH + 1], 1.0)
        ones1 = const.tile([1, 1], f32, name="ones1")
        nc.vector.memset(ones1, 1.0)

        nc.sync.dma_start(out=wq8, in_=wq8_d[:])
        nc.sync.dma_start(out=wk8, in_=wk8_d[:])
        nc.sync.dma_start(out=wv, in_=wv_d[:].rearrange("(c p) d -> p c d", p=128))
        nc.sync.dma_start(out=wo, in_=wo_d[:])

        # Phase 0: load host-pre-transposed x chunks (fp8 + bf16)
        for c in range(NCH):
            for tt in range(TOK // 2048):
                t0 = tt * 2048
                nc.sync.dma_start(
                    out=x8[:, c, t0 : t0 + 2048], in_=x8_d[c, :, t0 : t0 + 2048]
                )
                nc.sync.dma_start(
                    out=xb[:, c, t0 : t0 + 2048], in_=xb_d[c, :, t0 : t0 + 2048]
                )

        # Phase 1: projections. q/k in fp8 DoubleRow over feature-chunk
        # pairs; v in bf16 (fp8-projected V would leak its quantization
        # error directly into the output).
        for g in range(TOK // NQ):
            t0 = g * NQ
            qp = ps_s.tile([64, NQ], f32, tag="s", name="qp")
            for cp in range(2):
                nc.tensor.matmul(
                    qp, lhsT=wq8[:, cp, :, :],
                    rhs=x8[:, 2 * cp : 2 * cp + 2, t0 : t0 + NQ],
                    start=(cp == 0), stop=(cp == 1), perf_mode=DR,
                )
            nc.vector.tensor_copy(out=q8[:, 0, t0 : t0 + NQ], in_=qp[0:32, :])
            nc.vector.tensor_copy(out=q8[:, 1, t0 : t0 + NQ], in_=qp[32:64, :])

            kp = ps_s.tile([64, NQ], f32, tag="s", name="kp")
            for cp in range(2):
                nc.tensor.matmul(
                    kp, lhsT=wk8[:, cp, :, :],
                    rhs=x8[:, 2 * cp : 2 * cp + 2, t0 : t0 + NQ],
                    start=(cp == 0), stop=(cp == 1), perf_mode=DR,
                )
            nc.vector.tensor_copy(out=k8[:, 0, t0 : t0 + NQ], in_=kp[0:32, :])
            nc.vector.tensor_copy(out=k8[:, 1, t0 : t0 + NQ], in_=kp[32:64, :])

            for t in range(NQ // 128):
                tb = g * (NQ // 128) + t
                vp = ps_sm.tile([128, DH], f32, tag="o", name="vp")
                for c in range(NCH):
                    nc.tensor.matmul(
                        vp, lhsT=xb[:, c, tb * 128 : tb * 128 + 128], rhs=wv[:, c, :],
                        start=(c == 0), stop=(c == NCH - 1),
                    )
                nc.scalar.activation(out=vP[:, tb, 0:DH], in_=vp, func=CPY)

        # Phase 2+3: attention + output projection. Same interleaving as
        # the bf16 baseline: each group's epilogue (denominator transpose +
        # projection) is spread across the NEXT group's score loop.
        def emit_denT(pq0, pden):
            denT = ps_sm.tile([128, 4], f32, tag="o", name="denT")
            for t in range(NQ // 128):
                nc.tensor.matmul(
                    denT[:, t : t + 1],
                    lhsT=pden[0:1, t * 128 : (t + 1) * 128],
                    rhs=ones1, start=True, stop=True,
                )
            recT = sb_io.tile([128, 4], f32, name="recT")
            nc.vector.reciprocal(recT, denT)
            return recT

        def emit_fp_one(pq0, t, recT):
            tt0 = pq0 + t * 128
            fp = ps_sm.tile([128, D], f32, tag="o", name="fp")
            nc.tensor.matmul(
                fp, lhsT=oN[:, tt0 : tt0 + 128], rhs=wo, start=True, stop=True
            )
            ob = sb_io.tile([128, D], f32, name="ob")
            nc.vector.tensor_scalar_mul(ob, in0=fp, scalar1=recT[:, t : t + 1])
            nc.sync.dma_start(out=out_d[tt0 : tt0 + 128, :], in_=ob)

        pending = None  # (q0, den tile) of the previous group
        for b in range(B):
            for qg in range(N // NQ):
                q0 = b * N + qg * NQ
                o = ps_sm.tile([DH + 1, NQ], f32, tag="o", name="o")
                jb = 0
                recT = None
                # PV is deferred one chunk: the PE queue reads
                # [QK(g), PV(g-1), QK(g+1), PV(g), ...] so exp(g) on ScalarE
                # overlaps PV(g-1)/QK(g+1) instead of serializing the chunk.
                pv_queue = None  # (p tile, jb, gsz) awaiting emission
                def flush_pv(pv):
                    p, pjb, pgsz = pv
                    for i in range(pgsz):
                        jbg = b * NJB + pjb + i
                        nc.tensor.matmul(
                            o, lhsT=vP[:, jbg, :], rhs=p[:, i, :],
                            start=(pjb + i == 0), stop=(pjb + i == NJB - 1),
                        )
                for gi, gsz in enumerate(JGS):
                    s = ps_s.tile([128, 3, NQ], f32, tag="s", name="s")
                    for i in range(gsz):
                        j0 = b * N + (jb + i) * 128
                        nc.tensor.matmul(
                            s[:, i, :], lhsT=k8[:, :, j0 : j0 + 128],
                            rhs=q8[:, :, q0 : q0 + NQ], start=True, stop=True,
                            perf_mode=DR,
                        )
                    p = sb_p.tile([128, 3, NQ], bf16, name="p")
                    nc.scalar.activation(
                        out=p[:, 0:gsz, :], in_=s[:, 0:gsz, :], func=EXP, scale=SCALE
                    )
                    if pv_queue is not None:
                        flush_pv(pv_queue)
                    pv_queue = (p, jb, gsz)
                    jb += gsz
                    # previous group's epilogue, spread across this loop
                    if pending is not None:
                        if gi == 1:
                            recT = emit_denT(*pending)
                        elif gi in (3, 5, 7, 9):
                            emit_fp_one(pending[0], (gi - 3) // 2, recT)
                flush_pv(pv_queue)

                oc = o[0:DH, :]
                nc.vector.tensor_copy(out=oN[:, q0 : q0 + NQ], in_=oc)
                den = sb_io.tile([1, NQ], f32, name="den")
                nc.vector.tensor_copy(out=den, in_=o[DH : DH + 1, :])
                pending = (q0, den)

        recT = emit_denT(*pending)
        for t in range(NQ // 128):
            emit_fp_one(pending[0], t, recT)

    nc.compile()
    return nc


def make_in_maps(x, Wq, Wk, Wv, Wo):
    f8 = ml_dtypes.float8_e4m3
    bf16 = ml_dtypes.bfloat16
    xt = np.ascontiguousarray(x.reshape(TOK, D).T.reshape(NCH, 128, TOK))
    x8 = xt.astype(f8)
    xb = xt.astype(bf16)

    def wsplit(W, h):
        # [p, cp, i, d] = W.T[(2cp+i)*128+p, d] for the head's 64 columns
        wt = np.ascontiguousarray(W[h * DH : (h + 1) * DH, :].T)  # [512, 64]
        return np.ascontiguousarray(
            wt.reshape(2, 2, 128, DH).transpose(2, 0, 1, 3)
        ).astype(f8)

    in_maps = []
    for h in range(H):
        sl = slice(h * DH, (h + 1) * DH)
        in_maps.append(
            {
                "x8": x8,
                "xb": xb,
                "wq8": wsplit(Wq, h),
                "wk8": wsplit(Wk, h),
                "wv": np.ascontiguousarray(Wv[sl, :].T).astype(bf16),
                "wo": np.ascontiguousarray(Wo[:, sl].T).astype(bf16),
            }
        )
    return in_maps


def _install_ntff_shim():
    """The axon boot skips registering the NTFF profile hook when the image's
    antenv lacks axon_hooks; register an equivalent shim so trace=True works."""
    import types

    if "antenv.axon_hooks" in sys.modules:
        return
    try:
        from trn_agent_boot.trn_boot import _ntff_profile_via_ctypes

        hook = _ntff_profile_via_ctypes("/opt/axon/libaxon_pjrt.so")
    except Exception:
        hook = None
    mod = types.ModuleType("antenv.axon_hooks")
    mod.get_axon_ntff_profile_hook = lambda: hook
    sys.modules["antenv.axon_hooks"] = mod


def run(x, Wq, Wk, Wv, Wo, bo, trace=False):
    from concourse.bass_utils import run_bass_kernel_spmd

    if trace:
        _install_ntff_shim()

    nc = build_bass()
    in_maps = make_in_maps(x, Wq, Wk, Wv, Wo)
    res = run_bass_kernel_spmd(nc, in_maps, core_ids=list(range(H)), trace=trace)
    acc = np.zeros((TOK, D), dtype=np.float32)
    for r in res.results:
        acc += r["out"]
    acc += np.asarray(bo, dtype=np.float32)[None, :]
    return acc.reshape(B, N, D), res


def kernel(x, Wq, Wk, Wv, Wo, bo):
    out, _ = run(
        np.asarray(x, dtype=np.float32),
        np.asarray(Wq, dtype=np.float32),
        np.asarray(Wk, dtype=np.float32),
        np.asarray(Wv, dtype=np.float32),
        np.asarray(Wo, dtype=np.float32),
        np.asarray(bo, dtype=np.float32),
    )
    return out
